# revision 27
# baseline (speedup 1.0000x reference)
"""Causal self-attention (B=2, N=2048, D=768, H=12) on 8 Trainium2 NeuronCores.

Sharding: data-parallel over batch (2) x tensor-parallel over head groups (4),
3 heads per core. Each core computes, for its (batch, head-group):
  GEMM1: kT/qT (transposed) and v (natural) projections from xT,
  scores^T = k @ q^T per head, exp on ScalarE (fp16 out),
  AV with a ones-augmented V giving unnormalized sa + row sums,
  normalize, GEMM2 row-parallel -> yT partial (fp16).
All matmul operands are fp16 (fp32 PSUM accumulate). Host shards inputs, sums
the 4 per-batch partials (the "all-reduce"), and adds the output bias fold
(bproj + bkqv_v @ Wproj - exact because softmax rows sum to 1).

v3 changes vs v2 (trace-driven):
  - exp activations widened to 1024-col PSUM chunks (2-bank sc tiles,
    double-buffered): ~48 fewer ACT instructions, ~10us less ACT busy
  - k/q bias adds moved DVE->ACT (activation Identity with per-partition
    bias AP); they run in phase A where ACT is otherwise idle
  - normalize multiply reads the AV PSUM tile directly (no uav SBUF
    evacuation): ~13us less DVE busy
  - strips emitted head-interleaved; GEMM2 emitted per-oc-chunk
  - filler pop scans past not-yet-ready entries; emits a dummy matmul if
    nothing is ready (a starved PE re-throttles the HAM clock gate)

v4 changes:
  - reciprocal_approx_fast CANNOT read PSUM on hardware (probe: garbage
    results; CoreSim disagrees) - row sum goes through an SBUF copy again
  - ALL matmuls contract over the full 128 partitions: q is stored
    zero-padded per head (qz0=[q0;0], qz1=[0;q1], qz2=[q2;0]), kT1 and
    saT2/wp2 are zero-padded too. Strip/GEMM2 stream cost is unchanged
    (cost = moving cols), but the PE activity monitor appears to weigh
    active rows: K=64-heavy phases ran at K=4/8 clock (1.2GHz) even when
    gap-free, K=128-heavy phases at 2.4GHz. 0*0=0 keeps results exact.
  - GEMM2 readiness keyed off actual AV pop times + normalize latency;
    drain phase keeps the PE fed with dummies while normalize chains run.

v6 changes:
  - input DMAs split across both hardware DGE rings (SP + Activation
    engines) so the first GEMM1 inputs land earlier
  - exp ACT table preloaded via a tiny dummy activation at kernel start
    (the ~2.7us table load no longer delays the first real exp)
  - GEMM1 fused into the strip stream: only ci0-isl0 + the four ci1
    chains precede the first strip; remaining chains emitted inline just
    before the strips that consume them (ci0-isl k before group k's h0
    strip; all ci2 before the first h2 strip)
  - AV groups split into <=4-matmul sub-chunks with per-chunk readiness
    gates (act watermark of the newest strip the chunk reads). Chunks of
    one group chain-gate each other (PSUM accumulation stays in order);
    groups release two-behind (only 2 AV PSUM bufs). The final AV group
    is now ~75% done before its last exp lands, shrinking the tail.
  - pacing: smaller atomic fillers mean the PE never overshoots the ACT
    watermark by more than ~1us, so the strip/exp pipeline stays full.

Self-contained: hardcodes all shapes; no sibling imports.
"""

import os

import numpy as np

B, N, D = 2, 2048, 768
H, HD = 12, 64
HPC = 3           # heads per core
NG = 4            # head groups
NCORES = 8
P = 128
NJ = N // P       # 16 j-chunks (keys) per head
NISL = 4          # 512-query i-slices

_compiled = None  # cached compiled Bass module
last_exec_time_ns = None
last_results = None

N_WARMUP = 8      # 512-wide dummy matmuls bridging boot -> first GEMM1


def _build():
    import concourse.tile as tile
    import concourse.mybir as mybir
    from concourse import bacc

    f32 = mybir.dt.float32
    f16 = mybir.dt.float16
    MULT = mybir.AluOpType.mult
    EXP = mybir.ActivationFunctionType.Exp

    nc = bacc.Bacc(
        "TRN2", target_bir_lowering=False, debug=False, num_devices=NCORES
    )

    # packed DRAM layouts (see _host_prep)
    xT_d = nc.dram_tensor("xTp", [NISL, P, 6 * 512], f16, kind="ExternalInput").ap()
    wkq_d = nc.dram_tensor("wkqp", [3, P, 6 * 128], f16, kind="ExternalInput").ap()
    wv_d = nc.dram_tensor("wvp", [P, 6 * 192], f16, kind="ExternalInput").ap()
    wp01_d = nc.dram_tensor("wp01", [P, D], f16, kind="ExternalInput").ap()
    wp2_d = nc.dram_tensor("wp2", [P, D], f16, kind="ExternalInput").ap()
    bkq_d = nc.dram_tensor("bkq", [P, 4], f32, kind="ExternalInput").ap()
    ident_d = nc.dram_tensor("ident", [P, P], f16, kind="ExternalInput").ap()
    btri_d = nc.dram_tensor("btri", [P, P], f16, kind="ExternalInput").ap()
    yT_d = nc.dram_tensor("yT", [6, P, N], f16, kind="ExternalOutput").ap()

    xT_v = xT_d.rearrange("i p f -> p i f")      # [128, 4, 3072]
    wkq_v = wkq_d.rearrange("c p f -> p c f")    # [128, 3, 768]
    yT_v = yT_d.rearrange("o p f -> p o f")      # [128, 6, 2048]

    with tile.TileContext(nc) as tc:
        import contextlib

        ctx = contextlib.ExitStack()
        with ctx:
            const = ctx.enter_context(tc.tile_pool(name="const", bufs=1))
            big = ctx.enter_context(tc.tile_pool(name="bigbufs", bufs=1))
            work = ctx.enter_context(tc.tile_pool(name="work", bufs=3))
            ypool = ctx.enter_context(tc.tile_pool(name="ypool", bufs=3))
            # PSUM budget (8 banks): sc 2x[128,1024] (4) + gemm 2x[128,512]
            # (2) + av 2x[65,512] (2)
            psum_sc = ctx.enter_context(
                tc.tile_pool(name="psum_sc", bufs=2, space="PSUM")
            )
            psum_gemm = ctx.enter_context(
                tc.tile_pool(name="psum_gemm", bufs=2, space="PSUM")
            )
            psum_av = ctx.enter_context(
                tc.tile_pool(name="psum_av", bufs=2, space="PSUM")
            )

            # ---- SBUF tiles ----
            bkq_t = const.tile([P, 4], f32, name="bkq_t")
            wkq_ts = [const.tile([P, 768], f16, name=f"wkq_t{c}") for c in range(3)]
            xT_ts = [big.tile([P, 6 * 512], f16, name=f"xT_t{i}") for i in range(NISL)]
            ident_t = const.tile([P, P], f16, name="ident_t")
            btri_t = const.tile([P, P], f16, name="btri_t")
            wv_t = const.tile([P, 6 * 192], f16, name="wv_t")
            wp01_t = const.tile([P, D], f16, name="wp01_t")
            wp2_t = const.tile([P, D], f16, name="wp2_t")

            # input DMAs split across both hardware DGE rings (sync=SP and
            # scalar=ACT); per-ring issue order == priority order. xT is the
            # critical path (the q projections need all 4 slices before the
            # first strip finishes); ident/btri are tiny but gate the first
            # strip's mask; wv gates the v-projection fillers.
            HF = 1536
            nc.sync.dma_start(bkq_t[:], bkq_d)
            nc.sync.dma_start(wkq_ts[0][:], wkq_v[:, 0, :])
            nc.scalar.dma_start(xT_ts[0][:, HF:], xT_v[:, 0, HF:])
            nc.sync.dma_start(xT_ts[0][:, 0:HF], xT_v[:, 0, 0:HF])
            nc.scalar.dma_start(wkq_ts[1][:], wkq_v[:, 1, :])
            nc.sync.dma_start(ident_t[:], ident_d)
            nc.sync.dma_start(btri_t[:], btri_d)
            nc.sync.dma_start(xT_ts[1][:, 0:HF], xT_v[:, 1, 0:HF])
            nc.scalar.dma_start(xT_ts[1][:, HF:], xT_v[:, 1, HF:])
            nc.scalar.dma_start(xT_ts[2][:, HF:], xT_v[:, 2, HF:])
            nc.sync.dma_start(xT_ts[2][:, 0:HF], xT_v[:, 2, 0:HF])
            nc.scalar.dma_start(xT_ts[3][:, HF:], xT_v[:, 3, HF:])
            nc.sync.dma_start(xT_ts[3][:, 0:HF], xT_v[:, 3, 0:HF])
            nc.sync.dma_start(wv_t[:], wv_d)
            nc.scalar.dma_start(wkq_ts[2][:], wkq_v[:, 2, :])
            nc.sync.dma_start(wp01_t[:], wp01_d)
            nc.scalar.dma_start(wp2_t[:], wp2_d)

            # PE warmup on a zeroed scratch while the first inputs land
            wscr = const.tile([P, 512], f16, name="wscr")
            nc.vector.memset(wscr[:], 0.0)

            # preload the exp ACT table so the ~2.7us load overlaps the
            # input DMA wait instead of delaying the first real exp
            tpre = const.tile([1, 8], f16, name="tpre")
            nc.scalar.activation(tpre[:], wscr[0:1, 0:8], EXP, scale=0.125)

            def emit_dummy():
                wps = psum_gemm.tile([P, 512], f32, tag="ps512", name="wps")
                nc.tensor.matmul(
                    wps[:, 0:512], wscr[:, 0:128], wscr[:], start=True, stop=True
                )

            for _ in range(N_WARMUP):
                emit_dummy()

            # k tiles: kT0 = [k0; k1] on 128 partitions, kT1 = [k2; zeros].
            # q tiles zero-padded per head so strip matmuls contract K=128:
            # qz0 = [q0; 0], qz1 = [0; q1], qz2 = [q2; 0]. The zero halves
            # contribute 0 to the scores; full-K keeps the PE clock warm.
            kT0 = big.tile([P, N], f16, name="kT0")
            kT1 = big.tile([P, N], f16, name="kT1")
            qzs = [big.tile([P, N], f16, name=f"qz{h}") for h in range(HPC)]
            q2st = big.tile([P, N], f16, name="q2st")
            nc.vector.memset(kT1[64:128, :], 0.0)
            nc.vector.memset(qzs[0][64:128, :], 0.0)
            nc.vector.memset(qzs[1][0:64, :], 0.0)
            nc.vector.memset(qzs[2][64:128, :], 0.0)
            vaug = big.tile([P, NJ, HPC, 65], f16, name="vaug")
            nc.vector.memset(vaug[:, :, :, 64:65], 1.0)
            # saT: heads 0+1 packed on 128 partitions; head 2 zero-padded
            saT01s = [big.tile([P, 512], f16, name=f"saT01_{i}") for i in range(4)]
            saT2s = [big.tile([P, 512], f16, name=f"saT2_{i}") for i in range(4)]
            for i in range(4):
                nc.vector.memset(saT2s[i][64:128, :], 0.0)

            # ---- GEMM1 k/q: psum tile per (isl, ci) accumulated over dc ----
            # bias add + cast on ACT (idle during phase A)
            def emit_gemm1_kq(isl, ci):
                ps = psum_gemm.tile([P, 512], f32, tag="ps512", name="ps_kq")
                for dc in range(6):
                    nc.tensor.matmul(
                        ps[:, 0:512],
                        wkq_ts[ci][:, 128 * dc : 128 * dc + 128],
                        xT_ts[isl][:, 512 * dc : 512 * dc + 512],
                        start=(dc == 0),
                        stop=(dc == 5),
                    )
                sl = slice(512 * isl, 512 * isl + 512)
                if ci == 0:
                    nc.scalar.add(kT0[:, sl], ps[:, 0:512], bkq_t[:, 0:1])
                elif ci == 1:
                    nc.scalar.add(
                        qzs[0][0:64, sl], ps[0:64, 0:512], bkq_t[0:64, 1:2]
                    )
                    nc.scalar.add(
                        qzs[1][64:128, sl], ps[64:128, 0:512], bkq_t[64:128, 1:2]
                    )
                else:
                    nc.scalar.add(
                        kT1[0:64, sl], ps[0:64, 0:512], bkq_t[0:64, 2:3]
                    )
                    nc.scalar.add(
                        q2st[64:128, sl], ps[64:128, 0:512], bkq_t[64:128, 3:4]
                    )
                    nc.sync.dma_start(qzs[2][0:64, sl], q2st[64:128, sl])

            # ---- GEMM1 v: one psum tile per 128-query chunk ----
            def emit_gemm1_v(ic):
                ps = psum_gemm.tile([P, 512], f32, tag="ps512", name="ps_v")
                isl, k = divmod(ic, 4)
                for dc in range(6):
                    nc.tensor.matmul(
                        ps[:, 0:192],
                        xT_ts[isl][:, 512 * dc + 128 * k : 512 * dc + 128 * k + 128],
                        wv_t[:, 192 * dc : 192 * dc + 192],
                        start=(dc == 0),
                        stop=(dc == 5),
                    )
                nc.vector.tensor_copy(
                    out=vaug[:, ic, :, 0:64],
                    in_=ps[:, 0:192].rearrange("p (h d) -> p h d", h=HPC),
                )

            # ---- strips: scoresT + causal mask + exp (1024-wide chunks) ----
            all_strips = [[None] * NJ for _ in range(HPC)]

            def emit_strip(h, jc):
                kTc = kT0 if h < 2 else kT1
                qTc = qzs[h]
                i0 = 128 * jc
                W = N - i0
                strip = work.tile(
                    [P, W], f16, tag=f"expT{jc}", bufs=3, name=f"expT{jc}"
                )
                for c0 in range(0, W, 1024):
                    cw = min(1024, W - c0)
                    ps = psum_sc.tile([P, 1024], f32, tag="sc", name="ps_s")
                    for s0 in range(c0, c0 + cw, 512):
                        sw = min(512, W - s0)
                        chained = s0 == 0
                        nc.tensor.matmul(
                            ps[:, s0 - c0 : s0 - c0 + sw],
                            kTc[:, i0 : i0 + 128],
                            qTc[:, i0 + s0 : i0 + s0 + sw],
                            start=True,
                            stop=(not chained),
                        )
                        if chained:
                            # causal mask: accumulate -30000 above the diagonal
                            nc.tensor.matmul(
                                ps[:, 0:128], ident_t[:], btri_t[:],
                                start=False, stop=True,
                            )
                    nc.scalar.activation(
                        strip[:, c0 : c0 + cw], ps[:, 0:cw], EXP, scale=0.125
                    )
                all_strips[h][jc] = strip

            # ---- AV (emitted in sub-chunks) + normalize ----
            av_ps = {}

            def emit_av_part(h, iseg, jlo, jhi):
                strips = all_strips[h]
                jmax = 4 * iseg + 3
                if jlo == 0:
                    av_ps[(h, iseg)] = psum_av.tile(
                        [65, 512], f32, tag="av", name="ps2"
                    )
                ps2 = av_ps[(h, iseg)]
                for jc in range(jlo, jhi + 1):
                    off = 512 * iseg - 128 * jc
                    lo = max(0, off)
                    w = 512 - (lo - off)
                    nc.tensor.matmul(
                        ps2[0:65, 512 - w : 512],
                        vaug[:, jc, h, :],
                        strips[jc][:, lo : lo + w],
                        start=(jc == 0),
                        stop=(jc == jmax),
                    )
                if jhi != jmax:
                    return
                # row sum must bounce through SBUF: reciprocal_approx_fast
                # reads garbage from PSUM on hardware. The multiply below can
                # read PSUM directly. GpSimd runs ONLY partition_broadcast
                # (mixing op types forces library reloads).
                srow = work.tile([1, 512], f32, tag="srow", bufs=2, name="srow")
                nc.vector.tensor_copy(out=srow[:], in_=ps2[64:65, :])
                rrow = work.tile([1, 512], f32, tag="rrow", bufs=2, name="rrow")
                nc.vector.reciprocal_approx_fast(out=rrow[:], in_=srow[:])
                rbc = work.tile([64, 512], f32, tag="rbc", bufs=2, name="rbc")
                nc.gpsimd.partition_broadcast(rbc[:], rrow[:])
                if h == 0:
                    nc.vector.tensor_tensor(
                        saT01s[iseg][0:64, :], ps2[0:64, :], rbc[:], MULT
                    )
                elif h == 1:
                    st1 = work.tile([64, 512], f16, tag="st1", bufs=2, name="st1")
                    nc.vector.tensor_tensor(st1[:], ps2[0:64, :], rbc[:], MULT)
                    nc.sync.dma_start(saT01s[iseg][64:128, :], st1[:])
                else:
                    nc.vector.tensor_tensor(
                        saT2s[iseg][0:64, :], ps2[0:64, :], rbc[:], MULT
                    )

            # ---- GEMM2: heads 0+1 contract-128, head 2 contract-64 ----
            def emit_gemm2_oc(isl, oc):
                ps = psum_gemm.tile([P, 512], f32, tag="ps512", name="ps_y")
                nc.tensor.matmul(
                    ps[:, 0:512],
                    wp01_t[:, 128 * oc : 128 * oc + 128],
                    saT01s[isl][:],
                    start=True,
                    stop=False,
                )
                nc.tensor.matmul(
                    ps[:, 0:512],
                    wp2_t[:, 128 * oc : 128 * oc + 128],
                    saT2s[isl][:, :],
                    start=False,
                    stop=True,
                )
                yst = ypool.tile([P, 512], f16, tag="yst", name="yst")
                nc.vector.tensor_copy(out=yst[:], in_=ps[:, 0:512])
                nc.sync.dma_start(
                    yT_v[:, oc, 512 * isl : 512 * isl + 512], yst[:]
                )

            # ---- emission schedule ----
            # Costs in ns for the pacing model (warm clock).
            def strip_pe_cost(W):
                return W / 2.4 + 110 * ((W + 511) // 512) + 160

            def strip_act_cost(W):
                # calibrated: measured exp busy = 0.833ns/col + ~210ns/chunk
                return 0.833 * W + 210 * ((W + 1023) // 1024)

            # fillers: mutable [ready_gate, pe_cost, emit_fn] entries. The
            # gate compares against pe_t (emitted-PE-work watermark).
            fillers = []
            for ic in range(16):
                fillers.append([0.0, 580.0, lambda ic=ic: emit_gemm1_v(ic)])

            pe_t = 0.0    # PE-busy time emitted so far (phase B origin)
            act_t = 0.0   # ACT-busy time emitted so far
            SLACK = 3000.0
            NORM_DELAY = 3000.0   # AV drain -> saT ready (recip+bcast+mult)
            CHAIN = 6 * 512 / 2.4 + 120   # one GEMM1 chain on PE
            n_dummy = 0
            act_after = {}        # (h, jc) -> act_t watermark after its exp

            def emit_chain(isl, ci):
                # GEMM1 chain emitted inline in the strip stream
                nonlocal pe_t, act_t
                emit_gemm1_kq(isl, ci)
                pe_t += CHAIN
                act_t += 700.0 * (1 if ci == 0 else 2)

            def pop_fillers(budget, allow_dummy=True):
                # Pop ready fillers (scanning past not-yet-ready ones; AV
                # chunk/group ordering is enforced via dynamic gates). If
                # nothing is ready and a real deficit remains, emit a dummy
                # matmul: a starved PE re-throttles the HAM clock.
                nonlocal pe_t, n_dummy
                spent = 0.0
                while fillers and spent < budget:
                    for i, e in enumerate(fillers):
                        if e[0] <= pe_t:
                            fillers.pop(i)
                            e[2]()
                            pe_t += e[1]
                            spent += e[1]
                            break
                    else:
                        if allow_dummy and budget - spent > 400.0 and n_dummy < 150:
                            emit_dummy()
                            n_dummy += 1
                            pe_t += 215.0
                            spent += 215.0
                        else:
                            break
                return spent

            # AV groups: sub-chunks of <=4 jcs. Chunk i+1's gate opens when
            # chunk i pops (PSUM chain order); a group's first chunk opens
            # when the group two-before finished (2 AV PSUM bufs). GEMM2(g)
            # opens after AV(2,g)'s last chunk + normalize latency.
            av_ord = 0            # append ordinal
            av_group_done = set() # ordinals whose last chunk popped
            av_pending_first = {} # ordinal -> (entry, act_gate)
            gemm2_entries = {}

            def append_av_group(h, g):
                nonlocal av_ord
                n = av_ord
                av_ord += 1
                jmax = 4 * g + 3
                parts = [(jlo, min(jlo + 3, jmax)) for jlo in range(0, jmax + 1, 4)]
                entries = []
                for idx, (jlo, jhi) in enumerate(parts):
                    cols = sum(
                        min(512, 512 + 512 * g - 128 * jc)
                        for jc in range(jlo, jhi + 1)
                    )
                    cost = cols / 2.4 + 40.0 * (jhi - jlo + 1)
                    act_gate = act_after[(h, jhi)] + SLACK
                    last = jhi == jmax

                    def fn(h=h, g=g, jlo=jlo, jhi=jhi, idx=idx, n=n,
                           cost=cost, last=last):
                        emit_av_part(h, g, jlo, jhi)
                        if idx + 1 < len(entries):
                            # open the next chunk of this group
                            e2, gate2 = entries[idx + 1], gates[idx + 1]
                            e2[0] = gate2
                        if last:
                            av_group_done.add(n)
                            # release the group two ahead
                            if n + 2 in av_pending_first:
                                e2, gate2 = av_pending_first.pop(n + 2)
                                e2[0] = gate2
                            if h == HPC - 1:
                                for e2 in gemm2_entries[g]:
                                    e2[0] = pe_t + cost + NORM_DELAY

                    entries.append([float("inf"), cost, fn])
                gates = [act_after[(h, jhi)] + SLACK for (jlo, jhi) in parts]
                # first chunk: open if the group two-before is done
                if n < 2 or (n - 2) in av_group_done:
                    entries[0][0] = gates[0]
                else:
                    av_pending_first[n] = (entries[0], gates[0])
                if h == 0:
                    gemm2_entries[g] = [
                        [float("inf"), 620.0,
                         lambda g=g, oc=oc: emit_gemm2_oc(g, oc)]
                        for oc in range(6)
                    ]
                fillers.extend(entries)
                if h == HPC - 1:
                    fillers.extend(gemm2_entries[g])

            # ---- prefix: enough GEMM1 for the first strips ----
            emit_chain(0, 0)               # kT0 block 0 (g0, h0/h1)
            for isl in range(NISL):
                emit_chain(isl, 1)         # qz0/qz1 complete
            # reset pacing origin at the start of the strip stream
            pe_t = 0.0
            act_t = 0.0

            # remaining GEMM1 chains, emitted inline before their consumers:
            # ci0-isl k before group k's strips; all ci2 before any h2 strip
            inline_chains = {
                (0, 1): [(0, 2)], (0, 2): [(1, 2)],
                (0, 3): [(2, 2)], (0, 5): [(3, 2)],
                (0, 4): [(1, 0)], (0, 8): [(2, 0)], (0, 12): [(3, 0)],
            }

            for h in range(HPC):
                for jc in range(NJ):
                    for isl, ci in inline_chains.get((h, jc), []):
                        emit_chain(isl, ci)
                    W = N - 128 * jc
                    emit_strip(h, jc)
                    pe_t += strip_pe_cost(W)
                    act_t += strip_act_cost(W)
                    act_after[(h, jc)] = act_t
                    if jc % 4 == 3:
                        append_av_group(h, jc // 4)
                    # keep PE slightly ahead of ACT but not idle
                    pop_fillers(act_t - pe_t)

            # drain: keep popping; feed dummies while gates (normalize
            # chains) are still closed, then force-pop in order
            while fillers:
                if pop_fillers(1e9, allow_dummy=False) == 0.0:
                    if n_dummy < 150:
                        emit_dummy()
                        n_dummy += 1
                        pe_t += 215.0
                    else:
                        e = fillers.pop(0)
                        e[2]()
                        pe_t += e[1]

    nc.compile()
    return nc


def _host_prep(x, Wkqv, bkqv, Wproj, bproj):
    f16 = np.float16
    Wk = Wkqv[:, 0:D]
    Wq = Wkqv[:, D : 2 * D]
    Wv = Wkqv[:, 2 * D : 3 * D]
    bk = bkqv[0:D]
    bq = bkqv[D : 2 * D]
    bv = bkqv[2 * D : 3 * D]
    out_bias = (bproj + bv @ Wproj).astype(np.float32)  # softmax rows sum to 1

    ident = np.eye(P, dtype=f16)
    # btri[k, i] = -30000 where k > i: accumulated into scoresT diag blocks,
    # exp((s - 30000) * 0.125) underflows to exactly 0 in fp16.
    btri = (np.tril(np.full((P, P), -30000.0, np.float32), -1)).astype(f16)

    in_maps = []
    for b in range(B):
        xT = x[b].T.astype(f16)                       # [768, 2048]
        # [isl, pi, dc*512 + c] = xT[128*dc + pi, 512*isl + c]
        xTp = np.ascontiguousarray(
            xT.reshape(6, P, NISL, 512).transpose(2, 1, 0, 3).reshape(NISL, P, 6 * 512)
        )
        for g in range(NG):
            hs = [HPC * g + i for i in range(HPC)]
            wk = [np.asarray(Wk[:, HD * h : HD * h + HD]) for h in hs]
            wq = [np.asarray(Wq[:, HD * h : HD * h + HD]) for h in hs]
            wv = [np.asarray(Wv[:, HD * h : HD * h + HD]) for h in hs]
            # column chunks: ci0 = k01, ci1 = q01, ci2 = k2|q2
            wkq = np.concatenate(
                [wk[0], wk[1], wq[0], wq[1], wk[2], wq[2]], axis=1
            ).astype(np.float32)                       # [768, 384]
            # [ci, pi, dc*128 + c] = wkq[128*dc + pi, 128*ci + c]
            wkqp = np.ascontiguousarray(
                wkq.reshape(6, P, 3, P).transpose(2, 1, 0, 3).reshape(3, P, 6 * P)
            ).astype(f16)
            wv_c = np.concatenate(wv, axis=1).astype(np.float32)   # [768, 192]
            # [pi, dc*192 + c] = wv_c[128*dc + pi, c]
            wvp = np.ascontiguousarray(
                wv_c.reshape(6, P, 192).transpose(1, 0, 2).reshape(P, 6 * 192)
            ).astype(f16)
            wp01 = np.concatenate(
                [Wproj[HD * hs[0] : HD * hs[0] + HD, :],
                 Wproj[HD * hs[1] : HD * hs[1] + HD, :]], axis=0
            ).astype(f16)                              # [128, 768]
            wp2 = np.zeros((P, D), f16)                # [128, 768], rows 64+ zero
            wp2[0:64, :] = Wproj[HD * hs[2] : HD * hs[2] + HD, :].astype(f16)
            bkq = np.zeros((P, 4), np.float32)
            bkq[:, 0] = np.concatenate(
                [bk[HD * hs[0] : HD * hs[0] + HD], bk[HD * hs[1] : HD * hs[1] + HD]]
            )
            bkq[:, 1] = np.concatenate(
                [bq[HD * hs[0] : HD * hs[0] + HD], bq[HD * hs[1] : HD * hs[1] + HD]]
            )
            bkq[0:64, 2] = bk[HD * hs[2] : HD * hs[2] + HD]
            bkq[64:128, 3] = bq[HD * hs[2] : HD * hs[2] + HD]
            in_maps.append(
                dict(xTp=xTp, wkqp=wkqp, wvp=wvp, wp01=wp01, wp2=wp2,
                     bkq=bkq, ident=ident, btri=btri)
            )
    return in_maps, out_bias


def kernel(x, Wkqv, bkqv, Wproj, bproj):
    global _compiled, last_exec_time_ns, last_results
    import concourse.bass_utils as bass_utils

    x = np.asarray(x, np.float32)
    Wkqv = np.asarray(Wkqv, np.float32)
    bkqv = np.asarray(bkqv, np.float32)
    Wproj = np.asarray(Wproj, np.float32)
    bproj = np.asarray(bproj, np.float32)

    if _compiled is None:
        _compiled = _build()
    nc = _compiled

    in_maps, out_bias = _host_prep(x, Wkqv, bkqv, Wproj, bproj)

    trace = os.environ.get("BASS_KERNEL_TRACE", "0") == "1"
    res = bass_utils.run_bass_kernel_spmd(
        nc, in_maps, core_ids=list(range(NCORES)), trace=trace
    )
    last_exec_time_ns = res.exec_time_ns
    last_results = res

    out = np.zeros((B, N, D), np.float32)
    for b in range(B):
        acc = np.zeros((D, N), np.float32)
        for g in range(NG):
            acc += res.results[b * NG + g]["yT"].reshape(D, N).astype(np.float32)
        out[b] = acc.T + out_bias
    return out
